# revision 6
# baseline (speedup 1.0000x reference)
"""Trainium2 Bass kernel for the light-field disparity cost-volume build.

Input  x:   (2, 16, 25, 128, 128) f32  (b, c, n=angRes^2, h, w)
Output:     (2, 16, 25, 9, 128, 128) f32  (b, c, n, D, h, w)

out[b,c,(a1,a2),d,y,x] = x[b,c,(a1,a2), y + d*(2-a1), x + d*(2-a2)]
(zero outside the image), d in [-4, 4].

Pure data movement. Sharding: the 32 (b*c) slices split 4-per-core over
8 NeuronCores (data parallel, no cross-core communication).

Per core: the 4*25 input views are staged in SBUF with image row ->
partition and, per (slice, view), a 144-element padded row chunk in the
free dimension ([8 zeros][128 row][8 zeros]) so that column shifts of
any disparity read zeros from the pad instead of needing clipping. Row
shifts are handled by clipping the partition window and writing the
missing rows from an SBUF zeros region.

DMA plan per core (all access patterns are <=3 dims as DMA requires):
  - 20 loads  (one per slice x view-row a1), SWDGE on gpsimd
  - 180 interior stores (one per slice x a1 x disparity; each covers the
    5 views of a view-row via a strided mid dim), HWDGE split across the
    sync + scalar queues
  - 32 zero-row stores (one per a1!=2 x disparity!=0, covering all 4
    slices x 5 views), SWDGE on gpsimd
"""

from contextlib import ExitStack

import numpy as np

import concourse.bass as bass
import concourse.mybir as mybir
from concourse.bass import AP
from concourse.bass_utils import run_bass_kernel_spmd

F32 = mybir.dt.float32

B, C, NV, H, W = 2, 16, 25, 128, 128
A = 5          # angular resolution
MIND, MAXD = -4, 4
D = MAXD - MIND + 1
NCORES = 8
NS = (B * C) // NCORES      # slices per core = 4

CHUNK = 144                 # padded row elems per (slice, view)
ZOFF = NS * NV * CHUNK      # zeros region offset in the free dim
ZLEN = 1024                 # zeros region elems per partition
PITCH = ZOFF + ZLEN         # SBUF free elems per partition

X_V = H * W                 # input view stride (elems)
O_T = H * W                 # output tile stride
O_V = D * O_T               # output view stride
O_S = NV * O_V              # output slice stride


def _build_nc():
    nc = bass.Bass()
    x = nc.dram_tensor("x", [NS, NV, H, W], F32, kind="ExternalInput")
    out = nc.dram_tensor("out", [NS, NV, D, H, W], F32, kind="ExternalOutput")
    xt, ot = x, out

    groups = [(s, a1) for s in range(NS) for a1 in range(A)]
    # interior stores in group order, alternating between the two HWDGE
    # queues (sync/scalar) for issue-rate balance
    shares = ([], [])
    for gi, (s, a1) in enumerate(groups):
        for d in range(MIND, MAXD + 1):
            shares[(gi * D + d - MIND) % 2].append((gi, s, a1, d))

    zero_jobs = [
        (a1, d)
        for a1 in range(A)
        for d in range(MIND, MAXD + 1)
        if d * (A // 2 - a1) != 0
    ]

    with (
        ExitStack() as stack,
        nc.sbuf_tensor([128, PITCH], F32) as buf,
        nc.semaphore("msem") as msem,
        nc.semaphore("s1") as s1,
        nc.semaphore("s2") as s2,
        nc.semaphore("zsem") as zsem,
        nc.Block() as block,
    ):
        # one semaphore per load group: DMA completions are out of order
        # across the 16 SDMA engines, so a shared cumulative counter can't
        # order "load g done" for a specific g
        lsems = [
            stack.enter_context(nc.semaphore(f"lsem{gi}"))
            for gi in range(len(groups))
        ]
        bt = buf

        @block.vector
        def _(vector):
            # zero the column pads + the zeros region. Chunk k's tail pad
            # and chunk k+1's head pad are one contiguous 16-elem run.
            vector.memset(AP(bt, 0, [[PITCH, 128], [1, 8]]), 0.0).then_inc(msem, 1)
            vector.memset(
                AP(bt, 136, [[PITCH, 128], [CHUNK, NS * NV - 1], [1, 16]]), 0.0
            ).then_inc(msem, 1)
            vector.memset(
                AP(bt, ZOFF - 8, [[PITCH, 128], [1, 8 + ZLEN]]), 0.0
            ).then_inc(msem, 1)

        @block.gpsimd
        def _(gpsimd):
            # loads: x[s, 5*a1:5*a1+5] -> per-(s,v) padded chunks
            for gi, (s, a1) in enumerate(groups):
                v0 = NV * s + A * a1
                gpsimd.dma_start(
                    out=AP(bt, CHUNK * v0 + 8, [[PITCH, 128], [CHUNK, A], [1, W]]),
                    in_=AP(xt, v0 * X_V, [[W, H], [X_V, A], [1, W]]),
                ).then_inc(lsems[gi], 16)

            gpsimd.wait_ge(msem, 3)
            # zero-row stores: rows shifted out of range, all 4 slices x
            # 5 views of a view-row at once
            for a1, d in zero_jobs:
                r = d * (A // 2 - a1)
                nz = abs(r)
                di = d - MIND
                dst_off = (A * a1 * D + di) * O_T + ((H - r) * W if r > 0 else 0)
                gpsimd.dma_start(
                    out=AP(ot, dst_off, [[O_S, NS], [O_V, A], [1, W * nz]]),
                    in_=AP(bt, ZOFF, [[PITCH, NS * A], [1, W * nz]]),
                ).then_inc(zsem, 16)
            gpsimd.wait_ge(zsem, 16 * len(zero_jobs))

        def store_stream(engine, share, sem):
            engine.wait_ge(msem, 3)
            done_groups = 0
            for gi, s, a1, d in share:
                if gi >= done_groups:
                    done_groups = gi + 1
                    engine.wait_ge(lsems[gi], 16)
                r = d * (A // 2 - a1)
                nr = H - abs(r)
                di = d - MIND
                v0 = NV * s + A * a1
                src_off = max(0, r) * PITCH + CHUNK * v0 + 8 + 2 * d
                dst_off = s * O_S + (A * a1 * D + di) * O_T + max(0, -r) * W
                engine.dma_start(
                    out=AP(ot, dst_off, [[W, nr], [O_V, A], [1, W]]),
                    in_=AP(bt, src_off, [[PITCH, nr], [CHUNK - d, A], [1, W]]),
                ).then_inc(sem, 16)
            engine.wait_ge(sem, 16 * len(share))

        @block.sync
        def _(sync):
            store_stream(sync, shares[0], s1)

        @block.scalar
        def _(scalar):
            store_stream(scalar, shares[1], s2)

    return nc


_NC = None


def _get_nc():
    global _NC
    if _NC is None:
        _NC = _build_nc()
    return _NC


def kernel(x: np.ndarray) -> np.ndarray:
    assert x.shape == (B, C, NV, H, W), x.shape
    xs = np.ascontiguousarray(x.astype(np.float32, copy=False)).reshape(
        B * C, NV, H, W
    )
    in_maps = [{"x": xs[NS * k : NS * (k + 1)]} for k in range(NCORES)]
    res = run_bass_kernel_spmd(_get_nc(), in_maps, core_ids=list(range(NCORES)))
    out = np.concatenate([r["out"] for r in res.results], axis=0)
    return out.reshape(B, C, NV, D, H, W)
